# revision 1
# baseline (speedup 1.0000x reference)
"""Trainium2 Bass kernel for nn_PersonalizedHeteroGNN (2-layer hetero GraphSAGE).

Self-contained: host-side graph preprocessing (permutation/sharding) + Bass/Tile
device program run SPMD on 8 NeuronCores via bass2jax, full inputs -> full output.

Design:
  - Node space partitioned into type-pure 128-node "virtual blocks", dealt
    degree-balanced across 8 cores (same static block/chunk structure per core).
  - Each core aggregates for its own destination blocks: per 128-edge chunk,
    an indirect DMA gathers the 128 source rows (fp32, 256B each) from a
    replicated node-feature table; a DVE is_equal one-hot + PE matmul performs
    the segment-sum into PSUM.
  - Mean = per-partition multiply by 1/deg; SAGE layer = Wl @ aggr + Wr @ x + b
    computed feature-major on PE; relu/bias on ACT during PSUM evacuation.
  - Between layers the per-core slices are AllGathered into a replicated table.
"""
import os
import numpy as np

import concourse.bacc as bacc
import concourse.tile as tile
import concourse.mybir as mybir
from concourse import bass
from concourse.bass_utils import run_bass_kernel_spmd
from concourse.masks import make_identity

N_CORES = 8
F = mybir.dt.float32


# ----------------------------------------------------------------- host prep

def _plan(P, U, B, C, S, src, dst, deg):
    """Deal nodes into type-pure 128-lane blocks, balanced by in-degree.

    Returns dict with the virtual layout and per-core padded chunk arrays.
    """
    sizes = [P, U, B, C, S]
    N = sum(sizes)
    nb = [max(1, -(-sz // (128 * N_CORES))) for sz in sizes]   # blocks/core/type
    NBC = sum(nb)                                              # blocks per core
    NV = NBC * 128                                             # nodes per core
    NVT = NV * N_CORES

    # global node -> (core, block_in_core, lane)
    vid = np.empty(N, np.int64)        # global -> virtual id (core*NV + blk*128 + lane)
    base = 0
    tblock0 = np.cumsum([0] + nb)[:-1]  # first block index of each type within a core
    for t, sz in enumerate(sizes):
        ids = np.arange(base, base + sz)
        order = np.argsort(-deg[ids], kind="stable")           # high degree first
        nblk = nb[t] * N_CORES
        g = np.arange(sz) % nblk                               # global block of type t
        lane = np.arange(sz) // nblk
        core = g % N_CORES
        blk = tblock0[t] + g // N_CORES
        vid[ids[order]] = core * NV + blk * 128 + lane
        base += sz

    vsrc = vid[src]
    vdst = vid[dst]
    dcore = vdst // NV
    dblk = (vdst % NV) // 128
    dlane = vdst % 128

    # order edges by (core, block, src) for locality
    gblk = dcore * NBC + dblk
    order = np.lexsort((vsrc, gblk))
    gblk_s = gblk[order]
    vsrc_s = vsrc[order]
    dlane_s = dlane[order]

    cnt = np.bincount(gblk_s, minlength=NBC * N_CORES).reshape(N_CORES, NBC)
    # chunks per block, static per type (max over all blocks of the type)
    K = np.ones(NBC, np.int64)
    for t in range(len(sizes)):
        b0, b1 = tblock0[t], tblock0[t] + nb[t]
        K[b0:b1] = max(1, -(-cnt[:, b0:b1].max() // 128))
    CT = int(K.sum())                                          # chunks per core
    cbase = np.cumsum([0] + list(K))[:-1]                      # chunk base per block

    # slot position of each edge inside the padded per-core stream
    edge_pos = np.zeros(len(gblk_s), np.int64)
    blk_off = np.zeros(NBC * N_CORES + 1, np.int64)
    blk_off[1:] = np.cumsum(cnt.ravel())
    within = np.arange(len(gblk_s)) - blk_off[gblk_s]
    core_s = gblk_s // NBC
    blk_s = gblk_s % NBC
    edge_pos = cbase[blk_s] * 128 + within                     # within core stream

    idx_arr = np.zeros((N_CORES, CT * 128), np.int32)          # gather indices
    dst_arr = np.full((N_CORES, CT * 128), 200.0, np.float32)  # one-hot codes
    for c in range(N_CORES):
        m = core_s == c
        idx_arr[c, edge_pos[m]] = vsrc_s[m].astype(np.int32)
        dst_arr[c, edge_pos[m]] = dlane_s[m].astype(np.float32)

    # device layout [128 lanes, CT chunks]
    idx_dev = idx_arr.reshape(N_CORES, CT, 128).transpose(0, 2, 1).copy()
    dst_dev = dst_arr.reshape(N_CORES, CT, 128).transpose(0, 2, 1).copy()

    return dict(
        sizes=sizes, nb=nb, NBC=NBC, NV=NV, NVT=NVT, vid=vid, K=K, CT=CT,
        cbase=cbase, tblock0=tblock0, idx_dev=idx_dev, dst_dev=dst_dev,
    )


# ------------------------------------------------------------ device program

def _build(cfg):
    NBC, NV, NVT, CT = cfg["NBC"], cfg["NV"], cfg["NVT"], cfg["CT"]
    K, cbase, nb = cfg["K"], cfg["cbase"], cfg["nb"]
    NPB = nb[0]                                 # product blocks per core
    NPc = NPB * 128                             # products per core (padded)

    nc = bacc.Bacc(None, target_bir_lowering=False, debug=False)

    # inputs (per-core content differs; names shared)
    t_idx = nc.dram_tensor("g_idx", [128, CT], mybir.dt.int32, kind="ExternalInput")
    t_dst = nc.dram_tensor("g_dst", [128, CT], F, kind="ExternalInput")
    t_rec = nc.dram_tensor("g_rec", [128, NBC], F, kind="ExternalInput")
    t_pxT = nc.dram_tensor("g_pxT", [384, NPc], F, kind="ExternalInput")
    t_emb = nc.dram_tensor("g_emb", [NV - NPc, 64], F, kind="ExternalInput")
    t_pW = nc.dram_tensor("g_pW", [384, 64], F, kind="ExternalInput")
    t_pb = nc.dram_tensor("g_pb", [64, 1], F, kind="ExternalInput")
    t_W1l = nc.dram_tensor("g_W1l", [64, 64], F, kind="ExternalInput")
    t_W1r = nc.dram_tensor("g_W1r", [64, 64], F, kind="ExternalInput")
    t_b1 = nc.dram_tensor("g_b1", [64, 1], F, kind="ExternalInput")
    t_W2l = nc.dram_tensor("g_W2l", [64, 32], F, kind="ExternalInput")
    t_W2r = nc.dram_tensor("g_W2r", [64, 32], F, kind="ExternalInput")
    t_b2 = nc.dram_tensor("g_b2", [32, 1], F, kind="ExternalInput")
    t_out = nc.dram_tensor("g_out", [NV, 32], F, kind="ExternalOutput")

    # internal DRAM
    x0_own = nc.dram_tensor("x0_own", [NV, 64], F)
    x1_own = nc.dram_tensor("x1_own", [NV, 64], F)
    x0_full = nc.dram_tensor("x0_full", [NVT, 64], F)
    x1_full = nc.dram_tensor("x1_full", [NVT, 64], F)

    rg = [list(range(N_CORES))]

    with tile.TileContext(nc) as tc:
        with (
            tc.tile_pool(name="const", bufs=1) as constp,
            tc.tile_pool(name="meta", bufs=1) as metap,
            tc.tile_pool(name="wts", bufs=1) as wtsp,
            tc.tile_pool(name="gat", bufs=8) as gatp,
            tc.tile_pool(name="oh", bufs=8) as ohp,
            tc.tile_pool(name="sb", bufs=4) as sbp,
            tc.tile_pool(name="sb2", bufs=4) as sbp2,
            tc.tile_pool(name="rhs", bufs=12) as rhsp,
            tc.tile_pool(name="agg_ps", bufs=2, space="PSUM") as aggps,
            tc.tile_pool(name="tr_ps", bufs=2, space="PSUM") as trps,
            tc.tile_pool(name="h_ps", bufs=2, space="PSUM") as hps,
            tc.tile_pool(name="o_ps", bufs=2, space="PSUM") as ops,
        ):
            ident = constp.tile([128, 128], F)
            make_identity(nc, ident[:])
            iota_i = constp.tile([128, 128], mybir.dt.int32)
            nc.gpsimd.iota(iota_i[:], pattern=[[1, 128]], base=0, channel_multiplier=0)
            iota = constp.tile([128, 128], F)
            nc.vector.tensor_copy(out=iota[:], in_=iota_i[:])

            idxs = metap.tile([128, CT], mybir.dt.int32)
            nc.sync.dma_start(out=idxs[:], in_=t_idx[:])
            dsts = metap.tile([128, CT], F)
            nc.sync.dma_start(out=dsts[:], in_=t_dst[:])
            recs = metap.tile([128, NBC], F)
            nc.sync.dma_start(out=recs[:], in_=t_rec[:])

            pW = []
            for k in range(3):
                w = wtsp.tile([128, 64], F, tag=f"pW{k}")
                nc.sync.dma_start(out=w[:], in_=t_pW[k * 128:(k + 1) * 128, :])
                pW.append(w)
            pb = wtsp.tile([64, 1], F, tag="pb")
            nc.sync.dma_start(out=pb[:], in_=t_pb[:])
            W1l = wtsp.tile([64, 64], F, tag="W1l")
            nc.sync.dma_start(out=W1l[:], in_=t_W1l[:])
            W1r = wtsp.tile([64, 64], F, tag="W1r")
            nc.sync.dma_start(out=W1r[:], in_=t_W1r[:])
            b1 = wtsp.tile([64, 1], F, tag="b1")
            nc.sync.dma_start(out=b1[:], in_=t_b1[:])
            W2l = wtsp.tile([64, 32], F, tag="W2l")
            nc.sync.dma_start(out=W2l[:], in_=t_W2l[:])
            W2r = wtsp.tile([64, 32], F, tag="W2r")
            nc.sync.dma_start(out=W2r[:], in_=t_W2r[:])
            b2 = wtsp.tile([32, 1], F, tag="b2")
            nc.sync.dma_start(out=b2[:], in_=t_b2[:])

            # ---------------- projection: x0 for own product blocks ----------
            for b in range(NPB):
                hp = hps.tile([64, 128], F, tag="hT")
                rr = []
                for k in range(3):
                    r = rhsp.tile([128, 128], F, tag="pxT")
                    nc.sync.dma_start(
                        out=r[:], in_=t_pxT[k * 128:(k + 1) * 128, b * 128:(b + 1) * 128])
                    rr.append(r)
                for k in range(3):
                    nc.tensor.matmul(out=hp[:], lhsT=pW[k][:], rhs=rr[k][:],
                                     start=(k == 0), stop=(k == 2))
                hT = sbp.tile([64, 128], F, tag="hT_sb")
                nc.scalar.activation(out=hT[:], in_=hp[:],
                                     func=mybir.ActivationFunctionType.Relu, bias=pb[:])
                tp = ops.tile([128, 64], F, tag="hout")
                nc.tensor.transpose(out=tp[:], in_=hT[:], identity=ident[:64, :64])
                hrow = sbp2.tile([128, 64], F, tag="hrow")
                nc.scalar.activation(out=hrow[:], in_=tp[:],
                                     func=mybir.ActivationFunctionType.Copy)
                nc.sync.dma_start(out=x0_own[b * 128:(b + 1) * 128, :], in_=hrow[:])

            # embeddings: bulk copy into the non-product rows
            nc.gpsimd.dma_start(out=x0_own[NPc:, :], in_=t_emb[:])

            if not os.environ.get("GNN_NO_COLL"):
                nc.gpsimd.collective_compute(
                    "AllGather", mybir.AluOpType.bypass, replica_groups=rg,
                    ins=[x0_own[:, :]], outs=[x0_full[:, :]])

            # ---------------- one GNN layer ---------------------------------
            def layer(x_full, x_own, Wl, Wr, bias, fo, relu, out_own):
                for b in range(NBC):
                    kb = int(K[b])
                    cb = int(cbase[b])
                    ap = aggps.tile([128, 64], F, tag="agg")
                    NO_G = os.environ.get("GNN_NO_GATHER")
                    NO_MM = os.environ.get("GNN_NO_MM")
                    for c in range(cb, cb + kb):
                        if NO_G:
                            g = None
                        else:
                            g = gatp.tile([128, 64], F, tag="gat")
                            nc.gpsimd.indirect_dma_start(
                                out=g[:], out_offset=None, in_=x_full[:],
                                in_offset=bass.IndirectOffsetOnAxis(ap=idxs[:, c:c + 1], axis=0))
                        if NO_MM:
                            if c == cb:
                                nc.vector.memset(ap[:], 0.0)
                            continue
                        oh = ohp.tile([128, 128], F, tag="oh")
                        nc.vector.tensor_tensor(
                            out=oh[:], in0=iota[:],
                            in1=dsts[:, c:c + 1].to_broadcast([128, 128]),
                            op=mybir.AluOpType.is_equal)
                        nc.tensor.matmul(out=ap[:], lhsT=oh[:],
                                         rhs=(iota[:, :64] if g is None else g[:]),
                                         start=(c == cb), stop=(c == cb + kb - 1))
                    # mean
                    am = sbp.tile([128, 64], F, tag="am")
                    nc.vector.tensor_tensor(
                        out=am[:], in0=ap[:],
                        in1=recs[:, b:b + 1].to_broadcast([128, 64]),
                        op=mybir.AluOpType.mult)
                    # own x rows (for the Wr term)
                    xb = sbp2.tile([128, 64], F, tag="xb")
                    nc.sync.dma_start(out=xb[:], in_=x_own[b * 128:(b + 1) * 128, :])
                    tA = trps.tile([64, 128], F, tag="tr")
                    nc.tensor.transpose(out=tA[:], in_=am[:], identity=ident[:])
                    aT = sbp.tile([64, 128], F, tag="aT")
                    nc.scalar.activation(out=aT[:], in_=tA[:],
                                         func=mybir.ActivationFunctionType.Copy)
                    tX = trps.tile([64, 128], F, tag="tr")
                    nc.tensor.transpose(out=tX[:], in_=xb[:], identity=ident[:])
                    xT = sbp2.tile([64, 128], F, tag="xT")
                    nc.scalar.activation(out=xT[:], in_=tX[:],
                                         func=mybir.ActivationFunctionType.Copy)
                    hp = hps.tile([fo, 128], F, tag="hT")
                    nc.tensor.matmul(out=hp[:], lhsT=Wl[:], rhs=aT[:], start=True, stop=False)
                    nc.tensor.matmul(out=hp[:], lhsT=Wr[:], rhs=xT[:], start=False, stop=True)
                    hT = sbp.tile([fo, 128], F, tag="hT_sb")
                    nc.scalar.activation(
                        out=hT[:], in_=hp[:],
                        func=(mybir.ActivationFunctionType.Relu if relu
                              else mybir.ActivationFunctionType.Identity),
                        bias=bias[:])
                    tp = ops.tile([128, fo], F, tag="hout")
                    nc.tensor.transpose(out=tp[:], in_=hT[:], identity=ident[:fo, :fo])
                    hrow = sbp2.tile([128, fo], F, tag="hrow")
                    nc.scalar.activation(out=hrow[:], in_=tp[:],
                                         func=mybir.ActivationFunctionType.Copy)
                    nc.sync.dma_start(out=out_own[b * 128:(b + 1) * 128, :], in_=hrow[:])

            if not os.environ.get("GNN_SKIP_LAYERS"):
                layer(x0_full, x0_own, W1l, W1r, b1, 64, True, x1_own)
            if not os.environ.get("GNN_NO_COLL"):
                nc.gpsimd.collective_compute(
                    "AllGather", mybir.AluOpType.bypass, replica_groups=rg,
                    ins=[x1_own[:, :]], outs=[x1_full[:, :]])
            if not os.environ.get("GNN_SKIP_LAYERS"):
                layer(x1_full, x1_own, W2l, W2r, b2, 32, False, t_out)
            else:
                # still write the output tensor so the NEFF has all outputs
                layer(x1_full, x1_own, W2l, W2r, b2, 32, False, t_out) if False else None
                zb = sbp2.tile([128, 32], F, tag="hrow")
                nc.vector.memset(zb[:], 0.0)
                for b in range(NBC):
                    nc.sync.dma_start(out=t_out[b * 128:(b + 1) * 128, :], in_=zb[:])

    nc.compile()
    return nc


# ------------------------------------------------------------------- driver

_PREV = {}
LAST_RUN_S = None


def kernel(product_x, user_emb, brand_emb, cat_emb, shop_emb,
           proj_W, proj_b, c1_Wl, c1_bl, c1_Wr, c2_Wl, c2_bl, c2_Wr,
           pb_src, pb_dst, pc_src, pc_dst, ps_src, ps_dst, up_src, up_dst):
    P, U, B, C, S = (product_x.shape[0], user_emb.shape[0], brand_emb.shape[0],
                     cat_emb.shape[0], shop_emb.shape[0])
    N = P + U + B + C + S
    off_u, off_b, off_c, off_s = P, P + U, P + U + B, P + U + B + C

    pb_d = pb_dst.astype(np.int64) + off_b
    pc_d = pc_dst.astype(np.int64) + off_c
    ps_d = ps_dst.astype(np.int64) + off_s
    up_s = up_src.astype(np.int64) + off_u
    src = np.concatenate([pb_src, pb_d, pc_src, pc_d, ps_src, ps_d, up_s, up_dst])
    dst = np.concatenate([pb_d, pb_src, pc_d, pc_src, ps_d, ps_src, up_dst, up_s])
    src = src.astype(np.int64)
    dst = dst.astype(np.int64)

    deg = np.bincount(dst, minlength=N)
    cfg = _plan(P, U, B, C, S, src, dst, deg)
    NV, NBC, NPB = cfg["NV"], cfg["NBC"], cfg["nb"][0]
    NPc = NPB * 128
    vid = cfg["vid"]

    recip = (1.0 / np.maximum(deg, 1)).astype(np.float32)

    # per-core tensors
    in_maps = []
    emb_all = np.concatenate([user_emb, brand_emb, cat_emb, shop_emb], axis=0)
    for c in range(N_CORES):
        # which global node sits at each of this core's lanes (or -1)
        lanes_prod = np.full(NPc, -1, np.int64)
        lanes_rest = np.full(NV - NPc, -1, np.int64)
        # invert vid for this core
        mine = np.where(vid // NV == c)[0]
        loc = vid[mine] % NV
        is_prod = loc < NPc
        lanes_prod[loc[is_prod]] = mine[is_prod]
        lanes_rest[loc[~is_prod] - NPc] = mine[~is_prod]

        pxT = np.zeros((384, NPc), np.float32)
        pm = lanes_prod >= 0
        pxT[:, pm] = product_x[lanes_prod[pm]].T
        emb = np.zeros((NV - NPc, 64), np.float32)
        rm = lanes_rest >= 0
        emb[rm] = emb_all[lanes_rest[rm] - P]

        rec2d = np.zeros((128, NBC), np.float32)
        lane_ids = np.full(NV, -1, np.int64)
        lane_ids[loc] = mine
        l2 = lane_ids.reshape(NBC, 128).T   # [128, NBC]
        ok = l2 >= 0
        rec2d[ok] = recip[l2[ok]]

        in_maps.append({
            "g_idx": cfg["idx_dev"][c],
            "g_dst": cfg["dst_dev"][c],
            "g_rec": rec2d,
            "g_pxT": pxT,
            "g_emb": emb,
            "g_pW": proj_W.astype(np.float32),
            "g_pb": proj_b.reshape(64, 1).astype(np.float32),
            "g_W1l": c1_Wl.astype(np.float32),
            "g_W1r": c1_Wr.astype(np.float32),
            "g_b1": c1_bl.reshape(64, 1).astype(np.float32),
            "g_W2l": c2_Wl.astype(np.float32),
            "g_W2r": c2_Wr.astype(np.float32),
            "g_b2": c2_bl.reshape(32, 1).astype(np.float32),
        })

    key = (P, U, B, C, S, cfg["CT"])
    if _PREV.get("key") == key:
        nc = _PREV["nc"]
    else:
        nc = _build(cfg)
        _PREV.update(key=key, nc=nc)

    import time as _time
    _t0 = _time.time()
    res = run_bass_kernel_spmd(nc, in_maps, core_ids=list(range(N_CORES)))
    global LAST_RUN_S
    LAST_RUN_S = _time.time() - _t0

    out_virt = np.concatenate([res.results[c]["g_out"] for c in range(N_CORES)], axis=0)
    return out_virt[vid]



# revision 11
# speedup vs baseline: 2.4060x; 2.4060x over previous
"""Trainium2 Bass kernel for nn_PersonalizedHeteroGNN (2-layer hetero GraphSAGE).

Self-contained: host-side graph preprocessing (permutation/sharding) + Bass/Tile
device program run SPMD on 8 NeuronCores via bass2jax, full inputs -> full output.

Design (v2 — transfer-optimized):
  - The end-to-end time is dominated by host<->device transfer, so inputs are
    quantized: product_x as int8 with per-row scale (dequant folded into a
    per-lane multiply after the projection), embeddings / feature tables /
    output as bf16. Edge one-hot codes travel as uint8.
  - Node space partitioned into type-pure 128-node "virtual blocks", dealt
    degree-balanced across 8 cores (same static block/chunk structure per core).
  - Each core aggregates for its own destination blocks: per 128-edge chunk,
    an indirect DMA gathers the 128 source rows (bf16, 128B each) from a
    replicated node-feature table; a DVE is_equal one-hot + PE matmul performs
    the segment-sum into PSUM (fp32 accumulate).
  - Mean = per-partition multiply by 1/deg; SAGE layer = Wl @ aggr + Wr @ x + b
    in fp32 on PE; relu/bias on ACT during PSUM evacuation.
  - Between layers the per-core slices are AllGathered into a replicated bf16
    table.
"""
import os
import numpy as np
import ml_dtypes

import concourse.bacc as bacc
import concourse.tile as tile
import concourse.mybir as mybir
from concourse import bass
from concourse.bass_utils import run_bass_kernel_spmd
from concourse.masks import make_identity

N_CORES = 8
F = mybir.dt.float32
BF = mybir.dt.bfloat16
I8 = mybir.dt.int8
U8 = mybir.dt.uint8
NPBF16 = ml_dtypes.bfloat16


# ----------------------------------------------------------------- host prep

def _plan(P, U, B, C, S, src, dst, deg):
    """Deal nodes into type-pure 128-lane blocks, balanced by in-degree.

    Returns dict with the virtual layout and per-core padded chunk arrays.
    """
    sizes = [P, U, B, C, S]
    N = sum(sizes)
    nb = [max(1, -(-sz // (128 * N_CORES))) for sz in sizes]   # blocks/core/type
    NBC = sum(nb)                                              # blocks per core
    NV = NBC * 128                                             # nodes per core
    NVT = NV * N_CORES

    # global node -> (core, block_in_core, lane)
    vid = np.empty(N, np.int64)        # global -> virtual id (core*NV + blk*128 + lane)
    base = 0
    tblock0 = np.cumsum([0] + nb)[:-1]  # first block index of each type within a core
    for t, sz in enumerate(sizes):
        ids = np.arange(base, base + sz)
        order = np.argsort(-deg[ids], kind="stable")           # high degree first
        nblk = nb[t] * N_CORES
        g = np.arange(sz) % nblk                               # global block of type t
        lane = np.arange(sz) // nblk
        core = g % N_CORES
        blk = tblock0[t] + g // N_CORES
        vid[ids[order]] = core * NV + blk * 128 + lane
        base += sz

    vsrc = vid[src]
    vdst = vid[dst]
    dcore = vdst // NV
    dblk = (vdst % NV) // 128
    dlane = vdst % 128

    # order edges by (core, block, src) for locality
    gblk = dcore * NBC + dblk
    order = np.lexsort((vsrc, gblk))
    gblk_s = gblk[order]
    vsrc_s = vsrc[order]
    dlane_s = dlane[order]

    cnt = np.bincount(gblk_s, minlength=NBC * N_CORES).reshape(N_CORES, NBC)
    # chunks per block, static per type (max over all blocks of the type)
    K = np.ones(NBC, np.int64)
    for t in range(len(sizes)):
        b0, b1 = tblock0[t], tblock0[t] + nb[t]
        K[b0:b1] = max(1, -(-cnt[:, b0:b1].max() // 128))
    CT = int(K.sum())                                          # chunks per core
    cbase = np.cumsum([0] + list(K))[:-1]                      # chunk base per block

    # slot position of each edge inside the padded per-core stream
    blk_off = np.zeros(NBC * N_CORES + 1, np.int64)
    blk_off[1:] = np.cumsum(cnt.ravel())
    within = np.arange(len(gblk_s)) - blk_off[gblk_s]
    core_s = gblk_s // NBC
    blk_s = gblk_s % NBC
    edge_pos = cbase[blk_s] * 128 + within                     # within core stream

    idx_arr = np.zeros((N_CORES, CT * 128), np.int32)          # gather indices
    dst_arr = np.full((N_CORES, CT * 128), 255, np.uint8)      # one-hot codes
    for c in range(N_CORES):
        m = core_s == c
        idx_arr[c, edge_pos[m]] = vsrc_s[m].astype(np.int32)
        dst_arr[c, edge_pos[m]] = dlane_s[m].astype(np.uint8)

    # device layout [128 lanes, CT chunks]
    idx_dev = idx_arr.reshape(N_CORES, CT, 128).transpose(0, 2, 1).copy()
    dst_dev = dst_arr.reshape(N_CORES, CT, 128).transpose(0, 2, 1).copy()

    return dict(
        sizes=sizes, nb=nb, NBC=NBC, NV=NV, NVT=NVT, vid=vid, K=K, CT=CT,
        cbase=cbase, tblock0=tblock0, idx_dev=idx_dev, dst_dev=dst_dev,
    )


# ------------------------------------------------------------ device program

def _build(cfg):
    NBC, NV, NVT, CT = cfg["NBC"], cfg["NV"], cfg["NVT"], cfg["CT"]
    K, cbase, nb = cfg["K"], cfg["cbase"], cfg["nb"]
    NPB = nb[0]                                 # product blocks per core
    NPc = NPB * 128                             # products per core (padded)

    nc = bacc.Bacc(None, target_bir_lowering=False, debug=False)

    # inputs (per-core content differs; names shared)
    t_idx = nc.dram_tensor("g_idx", [128, CT], mybir.dt.int32, kind="ExternalInput")
    t_dst = nc.dram_tensor("g_dst", [128, CT], U8, kind="ExternalInput")
    t_rec = nc.dram_tensor("g_rec", [128, NBC], F, kind="ExternalInput")
    t_px = nc.dram_tensor("g_px", [384, NPc], I8, kind="ExternalInput")
    t_ps = nc.dram_tensor("g_ps", [128, NPB], F, kind="ExternalInput")
    t_emb = nc.dram_tensor("g_emb", [NV - NPc, 64], BF, kind="ExternalInput")
    t_pW = nc.dram_tensor("g_pW", [384, 64], F, kind="ExternalInput")
    t_pb = nc.dram_tensor("g_pb", [128, 64], F, kind="ExternalInput")
    t_W1l = nc.dram_tensor("g_W1l", [64, 64], F, kind="ExternalInput")
    t_W1r = nc.dram_tensor("g_W1r", [64, 64], F, kind="ExternalInput")
    t_b1 = nc.dram_tensor("g_b1", [64, 1], F, kind="ExternalInput")
    t_W2l = nc.dram_tensor("g_W2l", [64, 32], F, kind="ExternalInput")
    t_W2r = nc.dram_tensor("g_W2r", [64, 32], F, kind="ExternalInput")
    t_b2 = nc.dram_tensor("g_b2", [32, 1], F, kind="ExternalInput")
    t_out = nc.dram_tensor("g_out", [NV, 32], BF, kind="ExternalOutput")

    # internal DRAM
    x0_own = nc.dram_tensor("x0_own", [NV, 64], BF)
    x1_own = nc.dram_tensor("x1_own", [NV, 64], BF)
    x0_full = nc.dram_tensor("x0_full", [NVT, 64], BF)
    x1_full = nc.dram_tensor("x1_full", [NVT, 64], BF)

    rg = [list(range(N_CORES))]

    with tile.TileContext(nc) as tc:
        with (
            tc.tile_pool(name="const", bufs=1) as constp,
            tc.tile_pool(name="meta", bufs=1) as metap,
            tc.tile_pool(name="wts", bufs=1) as wtsp,
            tc.tile_pool(name="gat", bufs=8) as gatp,
            tc.tile_pool(name="oh", bufs=8) as ohp,
            tc.tile_pool(name="sb", bufs=4) as sbp,
            tc.tile_pool(name="sb2", bufs=4) as sbp2,
            tc.tile_pool(name="rhs", bufs=12) as rhsp,
            tc.tile_pool(name="rhs8", bufs=12) as rhsp8,
            tc.tile_pool(name="agg_ps", bufs=2, space="PSUM") as aggps,
            tc.tile_pool(name="tr_ps", bufs=2, space="PSUM") as trps,
            tc.tile_pool(name="h_ps", bufs=2, space="PSUM") as hps,
            tc.tile_pool(name="o_ps", bufs=2, space="PSUM") as ops,
        ):
            ident = constp.tile([128, 128], F)
            make_identity(nc, ident[:])
            identb = constp.tile([128, 128], BF)
            nc.vector.tensor_copy(out=identb[:], in_=ident[:])
            iota_i = constp.tile([128, 128], mybir.dt.int32)
            nc.gpsimd.iota(iota_i[:], pattern=[[1, 128]], base=0, channel_multiplier=0)
            iotab = constp.tile([128, 128], BF)
            nc.vector.tensor_copy(out=iotab[:], in_=iota_i[:])

            idxs = metap.tile([128, CT], mybir.dt.int32)
            nc.sync.dma_start(out=idxs[:], in_=t_idx[:])
            dst8 = metap.tile([128, CT], U8)
            nc.sync.dma_start(out=dst8[:], in_=t_dst[:])
            dsts = metap.tile([128, CT], BF)
            nc.vector.tensor_copy(out=dsts[:], in_=dst8[:])
            recs = metap.tile([128, NBC], F)
            nc.sync.dma_start(out=recs[:], in_=t_rec[:])
            scl = metap.tile([128, NPB], F)
            nc.sync.dma_start(out=scl[:], in_=t_ps[:])

            pW = []
            for k in range(3):
                w = wtsp.tile([128, 64], F, tag=f"pW{k}")
                nc.sync.dma_start(out=w[:], in_=t_pW[k * 128:(k + 1) * 128, :])
                pW.append(w)
            btile = wtsp.tile([128, 64], F, tag="pb")
            nc.sync.dma_start(out=btile[:], in_=t_pb[:])
            W1l = wtsp.tile([64, 64], F, tag="W1l")
            nc.sync.dma_start(out=W1l[:], in_=t_W1l[:])
            W1r = wtsp.tile([64, 64], F, tag="W1r")
            nc.sync.dma_start(out=W1r[:], in_=t_W1r[:])
            b1 = wtsp.tile([64, 1], F, tag="b1")
            nc.sync.dma_start(out=b1[:], in_=t_b1[:])
            W2l = wtsp.tile([64, 32], F, tag="W2l")
            nc.sync.dma_start(out=W2l[:], in_=t_W2l[:])
            W2r = wtsp.tile([64, 32], F, tag="W2r")
            nc.sync.dma_start(out=W2r[:], in_=t_W2r[:])
            b2 = wtsp.tile([32, 1], F, tag="b2")
            nc.sync.dma_start(out=b2[:], in_=t_b2[:])

            # ---------------- projection: x0 for own product blocks ----------
            # h_row = relu(s_p * (q_p @ W) + b) written as bf16 rows.
            for b in range(NPB):
                hp = hps.tile([64, 128], F, tag="hT")
                for k in range(3):
                    r8 = rhsp8.tile([128, 128], I8, tag="px8")
                    nc.sync.dma_start(
                        out=r8[:], in_=t_px[k * 128:(k + 1) * 128, b * 128:(b + 1) * 128])
                    r = rhsp.tile([128, 128], F, tag="pxf")
                    nc.vector.tensor_copy(out=r[:], in_=r8[:])
                    nc.tensor.matmul(out=hp[:], lhsT=pW[k][:], rhs=r[:],
                                     start=(k == 0), stop=(k == 2))
                hT = sbp.tile([64, 128], BF, tag="hT_sb")
                nc.scalar.activation(out=hT[:], in_=hp[:],
                                     func=mybir.ActivationFunctionType.Copy)
                tp = ops.tile([128, 64], BF, tag="hout")
                nc.tensor.transpose(out=tp[:], in_=hT[:], identity=identb[:64, :64])
                t1 = sbp2.tile([128, 64], F, tag="t1")
                nc.vector.tensor_scalar(
                    out=t1[:], in0=tp[:], scalar1=scl[:, b:b + 1], scalar2=None,
                    op0=mybir.AluOpType.mult)
                t2 = sbp.tile([128, 64], F, tag="t2")
                nc.vector.tensor_tensor(out=t2[:], in0=t1[:], in1=btile[:],
                                        op=mybir.AluOpType.add)
                hrow = sbp2.tile([128, 64], BF, tag="hrow")
                nc.vector.tensor_scalar_max(hrow[:], t2[:], 0.0)
                nc.sync.dma_start(out=x0_own[b * 128:(b + 1) * 128, :], in_=hrow[:])

            # embeddings: bulk copy into the non-product rows (bf16 -> bf16)
            nc.gpsimd.dma_start(out=x0_own[NPc:, :], in_=t_emb[:])

            nc.gpsimd.collective_compute(
                "AllGather", mybir.AluOpType.bypass, replica_groups=rg,
                ins=[x0_own[:, :]], outs=[x0_full[:, :]])

            # ---------------- one GNN layer ---------------------------------
            def layer(x_full, x_own, Wl, Wr, bias, fo, relu, out_own):
                for b in range(NBC):
                    kb = int(K[b])
                    cb = int(cbase[b])
                    ap = aggps.tile([128, 64], F, tag="agg")
                    for c in range(cb, cb + kb):
                        g = gatp.tile([128, 64], BF, tag="gat")
                        nc.gpsimd.indirect_dma_start(
                            out=g[:], out_offset=None, in_=x_full[:],
                            in_offset=bass.IndirectOffsetOnAxis(ap=idxs[:, c:c + 1], axis=0))
                        oh = ohp.tile([128, 128], BF, tag="oh")
                        nc.vector.tensor_tensor(
                            out=oh[:], in0=iotab[:],
                            in1=dsts[:, c:c + 1].to_broadcast([128, 128]),
                            op=mybir.AluOpType.is_equal)
                        nc.tensor.matmul(out=ap[:], lhsT=oh[:], rhs=g[:],
                                         start=(c == cb), stop=(c == cb + kb - 1))
                    # mean
                    am = sbp.tile([128, 64], BF, tag="am")
                    nc.vector.tensor_tensor(
                        out=am[:], in0=ap[:],
                        in1=recs[:, b:b + 1].to_broadcast([128, 64]),
                        op=mybir.AluOpType.mult)
                    # own x rows (for the Wr term)
                    xb = sbp2.tile([128, 64], BF, tag="xb")
                    nc.sync.dma_start(out=xb[:], in_=x_own[b * 128:(b + 1) * 128, :])
                    tA = trps.tile([64, 128], BF, tag="tr")
                    nc.tensor.transpose(out=tA[:], in_=am[:], identity=identb[:])
                    aT = sbp.tile([64, 128], F, tag="aT")
                    nc.scalar.activation(out=aT[:], in_=tA[:],
                                         func=mybir.ActivationFunctionType.Copy)
                    tX = trps.tile([64, 128], BF, tag="tr")
                    nc.tensor.transpose(out=tX[:], in_=xb[:], identity=identb[:])
                    xT = sbp2.tile([64, 128], F, tag="xT")
                    nc.scalar.activation(out=xT[:], in_=tX[:],
                                         func=mybir.ActivationFunctionType.Copy)
                    hp = hps.tile([64, 128], F, tag="hT")
                    nc.tensor.matmul(out=hp[:fo, :], lhsT=Wl[:], rhs=aT[:], start=True, stop=False)
                    nc.tensor.matmul(out=hp[:fo, :], lhsT=Wr[:], rhs=xT[:], start=False, stop=True)
                    hT = sbp.tile([64, 128], BF, tag="hT_sb")
                    nc.scalar.activation(
                        out=hT[:fo, :], in_=hp[:fo, :],
                        func=(mybir.ActivationFunctionType.Relu if relu
                              else mybir.ActivationFunctionType.Identity),
                        bias=bias[:])
                    tp = ops.tile([128, 64], BF, tag="hout")
                    nc.tensor.transpose(out=tp[:, :fo], in_=hT[:fo, :], identity=identb[:fo, :fo])
                    hrow = sbp2.tile([128, 64], BF, tag="hrow")
                    nc.scalar.activation(out=hrow[:, :fo], in_=tp[:, :fo],
                                         func=mybir.ActivationFunctionType.Copy)
                    nc.sync.dma_start(out=out_own[b * 128:(b + 1) * 128, :], in_=hrow[:, :fo])

            layer(x0_full, x0_own, W1l, W1r, b1, 64, True, x1_own)
            nc.gpsimd.collective_compute(
                "AllGather", mybir.AluOpType.bypass, replica_groups=rg,
                ins=[x1_own[:, :]], outs=[x1_full[:, :]])
            layer(x1_full, x1_own, W2l, W2r, b2, 32, False, t_out)

    nc.compile()
    return nc


# ------------------------------------------------------------------- driver

_PREV = {}
LAST_RUN_S = None


def kernel(product_x, user_emb, brand_emb, cat_emb, shop_emb,
           proj_W, proj_b, c1_Wl, c1_bl, c1_Wr, c2_Wl, c2_bl, c2_Wr,
           pb_src, pb_dst, pc_src, pc_dst, ps_src, ps_dst, up_src, up_dst):
    P, U, B, C, S = (product_x.shape[0], user_emb.shape[0], brand_emb.shape[0],
                     cat_emb.shape[0], shop_emb.shape[0])
    N = P + U + B + C + S
    off_u, off_b, off_c, off_s = P, P + U, P + U + B, P + U + B + C

    pb_d = pb_dst.astype(np.int64) + off_b
    pc_d = pc_dst.astype(np.int64) + off_c
    ps_d = ps_dst.astype(np.int64) + off_s
    up_s = up_src.astype(np.int64) + off_u
    src = np.concatenate([pb_src, pb_d, pc_src, pc_d, ps_src, ps_d, up_s, up_dst])
    dst = np.concatenate([pb_d, pb_src, pc_d, pc_src, ps_d, ps_src, up_dst, up_s])
    src = src.astype(np.int64)
    dst = dst.astype(np.int64)

    deg = np.bincount(dst, minlength=N)
    cfg = _plan(P, U, B, C, S, src, dst, deg)
    NV, NBC, NPB = cfg["NV"], cfg["NBC"], cfg["nb"][0]
    NPc = NPB * 128
    vid = cfg["vid"]

    recip = (1.0 / np.maximum(deg, 1)).astype(np.float32)

    # int8 quantization of product_x with per-row scale
    px32 = product_x.astype(np.float32, copy=False)
    s_row = np.abs(px32).max(axis=1)
    s_row[s_row == 0] = 1.0
    q_all = np.round(px32 * (127.0 / s_row)[:, None]).astype(np.int8)
    s_row = (s_row / 127.0).astype(np.float32)

    # per-core tensors
    in_maps = []
    emb_all = np.concatenate([user_emb, brand_emb, cat_emb, shop_emb],
                             axis=0).astype(NPBF16)
    for c in range(N_CORES):
        # which global node sits at each of this core's lanes (or -1)
        lanes_prod = np.full(NPc, -1, np.int64)
        lanes_rest = np.full(NV - NPc, -1, np.int64)
        # invert vid for this core
        mine = np.where(vid // NV == c)[0]
        loc = vid[mine] % NV
        is_prod = loc < NPc
        lanes_prod[loc[is_prod]] = mine[is_prod]
        lanes_rest[loc[~is_prod] - NPc] = mine[~is_prod]

        px_q = np.zeros((384, NPc), np.int8)
        pm = lanes_prod >= 0
        px_q[:, pm] = q_all[lanes_prod[pm]].T
        ps = np.zeros(NPc, np.float32)
        ps[pm] = s_row[lanes_prod[pm]]
        ps = ps.reshape(NPB, 128).T.copy()          # [128 lanes, NPB]
        emb = np.zeros((NV - NPc, 64), NPBF16)
        rm = lanes_rest >= 0
        emb[rm] = emb_all[lanes_rest[rm] - P]

        rec2d = np.zeros((128, NBC), np.float32)
        lane_ids = np.full(NV, -1, np.int64)
        lane_ids[loc] = mine
        l2 = lane_ids.reshape(NBC, 128).T   # [128, NBC]
        ok = l2 >= 0
        rec2d[ok] = recip[l2[ok]]

        in_maps.append({
            "g_idx": cfg["idx_dev"][c],
            "g_dst": cfg["dst_dev"][c],
            "g_rec": rec2d,
            "g_px": px_q,
            "g_ps": ps,
            "g_emb": emb,
            "g_pW": proj_W.astype(np.float32),
            "g_pb": np.tile(proj_b.reshape(1, 64).astype(np.float32), (128, 1)),
            "g_W1l": c1_Wl.astype(np.float32),
            "g_W1r": c1_Wr.astype(np.float32),
            "g_b1": c1_bl.reshape(64, 1).astype(np.float32),
            "g_W2l": c2_Wl.astype(np.float32),
            "g_W2r": c2_Wr.astype(np.float32),
            "g_b2": c2_bl.reshape(32, 1).astype(np.float32),
        })

    key = (P, U, B, C, S, cfg["CT"])
    if _PREV.get("key") == key:
        nc = _PREV["nc"]
    else:
        nc = _build(cfg)
        _PREV.update(key=key, nc=nc)

    import time as _time
    _t0 = _time.time()
    res = run_bass_kernel_spmd(nc, in_maps, core_ids=list(range(N_CORES)))
    global LAST_RUN_S
    LAST_RUN_S = _time.time() - _t0

    out_virt = np.concatenate([res.results[c]["g_out"] for c in range(N_CORES)], axis=0)
    return out_virt[vid].astype(np.float32)
